# revision 29
# baseline (speedup 1.0000x reference)
"""Trainium2 Bass kernel for single-head attention model.

Reference computation (B=4, S=2048, E=1024, fp32):
    q = query @ Wq + bq;  k = key @ Wk + bk;  v = value @ Wv + bv
    scores = (q @ k^T) / sqrt(E)
    out = softmax(scores, axis=-1) @ v

Sharding: 8 cores; core c handles batch b = c // 2, query-row half
h = c % 2 (1024 q-rows). K/V projections for the full batch are
computed redundantly on both cores of a pair (no collectives).

v5 strategy:
  - inputs/weights bf16; host packs everything partition-major so each
    phase needs a handful of large DMAs; weights m-major
  - all input tiles in top-level pools (no SBUF-reuse anti-deps) and
    ONE shared PSUM rotation for every 8-matmul accumulation group so
    no phase boundary ever stalls the PE
  - phase order K-proj -> V-proj -> Q-proj -> attention
  - first K-proj block processed in two 256-wide halves so the first
    matmul starts as early as possible
  - QT/KT written as fp8e4 in k-tile pairs; the scores matmul runs
    perf_mode=DoubleRow (K=256 per instruction, ~2x PE rate)
  - attention: exp via ACT (bf16), per-tile running-sum adds on DVE
    interleaved with the score groups, normalization via ones-matmul
    partition reduce + reciprocal applied on psum eviction
  - out written [128, 8, E] partition-major, per-512-slice DMAs
"""

import sys

sys.path.insert(0, "/opt/trn_rl_repo")

from contextlib import ExitStack

import numpy as np
import ml_dtypes

import concourse.bass as bass
import concourse.mybir as mybir
import concourse.tile as tile
from concourse import bacc, bass_utils

F32R = mybir.dt.float32r
F32 = mybir.dt.float32
BF = mybir.dt.bfloat16
FP8 = mybir.dt.float8e4
AF = mybir.ActivationFunctionType
DR = mybir.MatmulPerfMode.DoubleRow
NPBF = ml_dtypes.bfloat16

B, S, E = 4, 2048, 1024
N_CORES = 8
SQ = S // 2          # q rows per core
SK = S               # kv rows per core
BQ = 512             # s_q block width in phase D
NBLK = SQ // BQ      # 2 blocks
EK = E // 128        # 8 contraction tiles over e
MK = SK // 128       # 16 s_k tiles
INV_SCALE = 1.0 / float(np.sqrt(E))

USE_FP8_SCORES = True

_cached = {}


def _build():
    nc = bacc.Bacc("TRN2", target_bir_lowering=False, debug=False,
                   num_devices=N_CORES)

    kq_dt = FP8 if USE_FP8_SCORES else BF

    # x layouts: [128, nb, 8(k), 512]
    xqh = nc.dram_tensor("xqh", [128, SQ // 512, EK, 512], BF,
                         kind="ExternalInput").ap()
    xkh = nc.dram_tensor("xkh", [128, SK // 512, EK, 512], BF,
                         kind="ExternalInput").ap()
    xvh = nc.dram_tensor("xvh", [128, SK // 512, EK, 512], BF,
                         kind="ExternalInput").ap()
    # wq/wk: [128, 8(m), 8(k), 128] m-major; wv: [128, 8(k), 1024]
    wqh = nc.dram_tensor("wqh", [128, E * EK], BF, kind="ExternalInput").ap()
    wkh = nc.dram_tensor("wkh", [128, E * EK], BF, kind="ExternalInput").ap()
    wvh = nc.dram_tensor("wvh", [128, E * EK], BF, kind="ExternalInput").ap()
    # consts: cols [2:10]=bq, [10:18]=bk  (f32)
    csth = nc.dram_tensor("csth", [128, 18], F32, kind="ExternalInput").ap()
    ones_in = nc.dram_tensor("ones_in", [128, 2], F32R, kind="ExternalInput").ap()
    # bv broadcast to all partitions (bf16)
    bvh = nc.dram_tensor("bvh", [128, E], BF, kind="ExternalInput").ap()
    # out row-tiles g=0..7 partition-major: [p, g, e] = out[g*128+p, e]
    out = nc.dram_tensor("out", [128, SQ // 128, E], F32,
                         kind="ExternalOutput").ap()

    with tile.TileContext(nc) as tc, ExitStack() as top:
        # ---- pools (all top-level) ----
        consts = top.enter_context(tc.tile_pool(name="consts", bufs=1))
        ktpool = top.enter_context(tc.tile_pool(name="ktpool", bufs=1))
        qtpool = top.enter_context(tc.tile_pool(name="qtpool", bufs=1))
        vpool = top.enter_context(tc.tile_pool(name="vpool", bufs=1))
        expp = top.enter_context(tc.tile_pool(name="expp", bufs=1))
        wkp = top.enter_context(tc.tile_pool(name="wkp", bufs=1))
        wqp = top.enter_context(tc.tile_pool(name="wqp", bufs=1))
        wvp = top.enter_context(tc.tile_pool(name="wvp", bufs=1))
        xqp = top.enter_context(tc.tile_pool(name="xqp", bufs=1))
        xkp = top.enter_context(tc.tile_pool(name="xkblk", bufs=2))
        xvp = top.enter_context(tc.tile_pool(name="xvblk", bufs=2))
        partp = top.enter_context(tc.tile_pool(name="partp", bufs=1))
        outp = top.enter_context(tc.tile_pool(name="outp", bufs=2))
        sumsp = top.enter_context(tc.tile_pool(name="sumsp", bufs=4))
        psMain = top.enter_context(tc.tile_pool(name="psMain", bufs=4,
                                                space="PSUM"))
        psO = top.enter_context(tc.tile_pool(name="psO", bufs=3, space="PSUM"))
        psSum = top.enter_context(tc.tile_pool(name="psSum", bufs=1,
                                               space="PSUM"))

        cst = consts.tile([128, 18], F32)
        ones_t = consts.tile([128, 2], F32R)
        bq_t = cst[:, 2:10]
        bk_t = cst[:, 10:18]

        # KT/QT as k-tile PAIRS for DoubleRow: [128, 2, cols]
        kt_pair = [ktpool.tile([128, 2, SK], kq_dt, tag=f"kt{j}", name=f"kt{j}")
                   for j in range(EK // 2)]
        qt_pair = [qtpool.tile([128, 2, SQ], kq_dt, tag=f"qt{j}", name=f"qt{j}")
                   for j in range(EK // 2)]
        v_tiles = [vpool.tile([128, E], BF, tag=f"v{m}", name=f"v{m}")
                   for m in range(MK)]
        exp_tiles = [expp.tile([128, BQ], BF, tag=f"exp{m}", name=f"exp{m}")
                     for m in range(MK)]
        wk_sb = wkp.tile([128, E * EK], BF)
        wq_sb = wqp.tile([128, E * EK], BF)
        wv_sb = wvp.tile([128, E * EK], BF)
        xq_sb = xqp.tile([128, SQ // 512, EK, 512], BF)

        # hoist the ACT Exp table load off the critical path (depends only
        # on a memset so it can run immediately)
        actsrc = consts.tile([1, 2], F32)
        nc.gpsimd.memset(actsrc[:], 0.0)
        actwarm = consts.tile([1, 2], F32)
        nc.scalar.activation(actwarm[:], actsrc[:], AF.Exp)

        # ======== phase B: KT = Wk^T @ xkT + bk ========
        # DMA order follows the compute critical path: m0 weights, first
        # half-block, then weight slices interleaved just ahead of use
        nc.sync.dma_start(wk_sb[:, 0:E], wkh[:, 0:E])
        xk_first = xkp.tile([128, EK, 512], BF, tag="xkb", name="xkb0")
        nc.sync.dma_start(xk_first[:, :, 0:256], xkh[:, 0, :, 0:256])
        nc.sync.dma_start(wk_sb[:, E:2 * E], wkh[:, E:2 * E])
        nc.sync.dma_start(xk_first[:, :, 256:512], xkh[:, 0, :, 256:512])
        nc.sync.dma_start(cst[:], csth)
        for m in range(2, EK):
            nc.sync.dma_start(wk_sb[:, m * E:(m + 1) * E],
                              wkh[:, m * E:(m + 1) * E])
        nc.sync.dma_start(ones_t[:], ones_in)
        bv_bc = consts.tile([128, E], BF)
        for nb in range(SK // 512):
            if nb == 0:
                xk_blk = xk_first
                # two 256-wide halves so the first matmul starts early
                spans = [(0, 256), (256, 512)]
            else:
                xk_blk = xkp.tile([128, EK, 512], BF, tag="xkb",
                                  name=f"xkb{nb}")
                nc.sync.dma_start(xk_blk[:], xkh[:, nb])
                spans = [(0, 512)]
            for s0, s1 in spans:
                for m in range(EK):
                    w = s1 - s0
                    ps = psMain.tile([128, 512], F32, tag="ps")
                    for k in range(EK):
                        nc.tensor.matmul(
                            ps[:, 0:w],
                            wk_sb[:, m * E + k * 128:m * E + (k + 1) * 128],
                            xk_blk[:, k, s0:s1],
                            start=(k == 0), stop=(k == EK - 1))
                    j, kk = divmod(m, 2)
                    nc.vector.tensor_scalar_add(
                        kt_pair[j][:, kk:kk + 1, nb * 512 + s0:nb * 512 + s1],
                        ps[:, 0:w], bk_t[:, m:m + 1])

        # ======== phase A: V = xvT^T @ Wv + bv ========
        nc.sync.dma_start(bv_bc[:], bvh)
        nc.sync.dma_start(wv_sb[:], wvh)
        for mb in range(SK // 512):
            xv_blk = xvp.tile([128, EK, 512], BF, tag="xvb", name=f"xvb{mb}")
            nc.sync.dma_start(xv_blk[:], xvh[:, mb])
            for n in range(E // 512):
                for i in range(4):
                    ps = psMain.tile([128, 512], F32, tag="ps")
                    for k in range(EK):
                        nc.tensor.matmul(
                            ps[:],
                            xv_blk[:, k, i * 128:(i + 1) * 128],
                            wv_sb[:, k * E + n * 512:k * E + (n + 1) * 512],
                            start=(k == 0), stop=(k == EK - 1))
                    nc.vector.tensor_add(
                        v_tiles[mb * 4 + i][:, n * 512:(n + 1) * 512],
                        ps[:], bv_bc[:, n * 512:(n + 1) * 512])

        # ======== phase C: QT = Wq^T @ xqT + bq ========
        nc.sync.dma_start(wq_sb[:, 0:2 * E], wqh[:, 0:2 * E])
        nc.sync.dma_start(xq_sb[:, 0], xqh[:, 0])
        nc.sync.dma_start(wq_sb[:, 2 * E:8 * E], wqh[:, 2 * E:8 * E])
        nc.sync.dma_start(xq_sb[:, 1], xqh[:, 1])
        for n in range(SQ // 512):
            for m in range(EK):
                ps = psMain.tile([128, 512], F32, tag="ps")
                for k in range(EK):
                    nc.tensor.matmul(
                        ps[:],
                        wq_sb[:, m * E + k * 128:m * E + (k + 1) * 128],
                        xq_sb[:, n, k],
                        start=(k == 0), stop=(k == EK - 1))
                j, kk = divmod(m, 2)
                nc.vector.tensor_scalar_add(
                    qt_pair[j][:, kk:kk + 1, n * 512:(n + 1) * 512],
                    ps[:], bq_t[:, m:m + 1])

        # ======== phase D: attention, blocked over s_q ========
        for blk in range(NBLK):
            q0 = blk * BQ
            # scoresT = KT^T @ QT_blk; exp (bf16); running sum on DVE
            part = partp.tile([128, BQ], F32, tag="part")
            part_r = partp.tile([128, BQ], F32R, tag="part_r")
            for m in range(MK):
                ps = psMain.tile([128, 512], F32, tag="ps")
                if USE_FP8_SCORES:
                    for j in range(EK // 2):
                        nc.tensor.matmul(
                            ps[:],
                            kt_pair[j][:, 0:2, m * 128:(m + 1) * 128],
                            qt_pair[j][:, 0:2, q0:q0 + BQ],
                            start=(j == 0), stop=(j == EK // 2 - 1),
                            perf_mode=DR)
                else:
                    for j in range(EK // 2):
                        for kk in range(2):
                            nc.tensor.matmul(
                                ps[:],
                                kt_pair[j][:, kk, m * 128:(m + 1) * 128],
                                qt_pair[j][:, kk, q0:q0 + BQ],
                                start=(j == 0 and kk == 0),
                                stop=(j == EK // 2 - 1 and kk == 1))
                nc.scalar.activation(exp_tiles[m][:], ps[:], AF.Exp,
                                     scale=INV_SCALE)
                if m == 1:
                    nc.vector.tensor_add(part[:], exp_tiles[0][:],
                                         exp_tiles[1][:])
                elif m == MK - 1:
                    nc.vector.tensor_add(part_r[:], part[:], exp_tiles[m][:])
                elif m > 1:
                    nc.vector.tensor_add(part[:], part[:], exp_tiles[m][:])

            # out_unnorm[s_q, e] = expT^T @ V ; sums via ones-matmul.
            # The pssum matmul is emitted AFTER the first psO group's
            # matmuls so the PE never waits on the DVE running-sum chain.
            for mi in range(BQ // 128):
                g = blk * (BQ // 128) + mi
                ot = outp.tile([128, E], F32, tag="ot")
                recip = sumsp.tile([128, 1], F32, tag="recip")
                last = (blk == NBLK - 1 and mi == BQ // 128 - 1)
                for n in range(E // 512):
                    pso = psO.tile([128, 512], F32, tag="psO")
                    for m in range(MK):
                        nc.tensor.matmul(
                            pso[:],
                            exp_tiles[m][:, mi * 128:(mi + 1) * 128],
                            v_tiles[m][:, n * 512:(n + 1) * 512],
                            start=(m == 0), stop=(m == MK - 1))
                    if n == 0:
                        pssum = psSum.tile([128, 2], F32, tag="pssum")
                        off = -20 if mi == 0 else 0
                        with tc.high_priority(offset=off):
                            nc.tensor.matmul(
                                pssum[:],
                                part_r[:, mi * 128:(mi + 1) * 128],
                                ones_t[:], start=True, stop=True)
                            nc.vector.reciprocal(recip[:], pssum[:, 0:1])
                    if last and n == E // 512 - 1:
                        # split the final eviction/DMA to shorten the tail
                        for h2 in range(2):
                            sl = slice(n * 512 + h2 * 256,
                                       n * 512 + (h2 + 1) * 256)
                            nc.vector.tensor_scalar_mul(
                                ot[:, sl], pso[:, h2 * 256:(h2 + 1) * 256],
                                recip[:])
                            nc.sync.dma_start(out[:, g, sl], ot[:, sl])
                    else:
                        nc.vector.tensor_scalar_mul(
                            ot[:, n * 512:(n + 1) * 512], pso[:], recip[:])
                        nc.sync.dma_start(
                            out[:, g, n * 512:(n + 1) * 512],
                            ot[:, n * 512:(n + 1) * 512])

    nc.compile()
    return nc


def _get_nc():
    if "nc" not in _cached:
        _cached["nc"] = _build()
    return _cached["nc"]


def _pack_w_mmajor(W):
    # [128, 8(m), 8(k), 128]: [p, m, k, c] = W[k*128+p, m*128+c]
    return np.ascontiguousarray(
        W.reshape(EK, 128, EK, 128).transpose(1, 2, 0, 3).reshape(128, E * EK)
        .astype(NPBF))


def _pack_w_kmajor(W):
    # [128, 8(k), 1024]: [p, k, c] = W[k*128+p, c]
    return np.ascontiguousarray(
        W.reshape(EK, 128, E).transpose(1, 0, 2).reshape(128, E * EK)
        .astype(NPBF))


def _pack_x(x, blk):
    # [128, nb, 8(k), blk]: [p, nb, k, s] = x[nb*blk+s, k*128+p]
    nb = x.shape[0] // blk
    return np.ascontiguousarray(
        x.reshape(nb, blk, EK, 128).transpose(3, 0, 2, 1).astype(NPBF))


def kernel(query, key, value, Wq, bq, Wk, bk, Wv, bv, **kw):
    query = np.asarray(query, dtype=np.float32)
    key = np.asarray(key, dtype=np.float32)
    value = np.asarray(value, dtype=np.float32)
    wq_h = _pack_w_mmajor(np.asarray(Wq, dtype=np.float32))
    wk_h = _pack_w_mmajor(np.asarray(Wk, dtype=np.float32))
    wv_h = _pack_w_kmajor(np.asarray(Wv, dtype=np.float32))
    bq = np.asarray(bq, dtype=np.float32)
    bk = np.asarray(bk, dtype=np.float32)
    bv = np.asarray(bv, dtype=np.float32)

    cst_h = np.empty((128, 18), dtype=np.float32)
    cst_h[:, 0:2] = 1.0
    cst_h[:, 2:10] = bq.reshape(EK, 128).T
    cst_h[:, 10:18] = bk.reshape(EK, 128).T
    bv_h = np.ascontiguousarray(
        np.broadcast_to(bv.reshape(1, E), (128, E)).astype(NPBF))

    xk_h = {b: _pack_x(key[b], 512) for b in range(B)}
    xv_h = {b: _pack_x(value[b], 512) for b in range(B)}

    in_maps = []
    for c in range(N_CORES):
        b, h = divmod(c, 2)
        xq_h = _pack_x(query[b, h * SQ:(h + 1) * SQ, :], 512)
        in_maps.append({
            "xqh": xq_h, "xkh": xk_h[b], "xvh": xv_h[b],
            "wqh": wq_h, "wkh": wk_h, "wvh": wv_h,
            "csth": cst_h, "bvh": bv_h,
            "ones_in": np.ones((128, 2), dtype=np.float32),
        })

    nc = _get_nc()
    res = bass_utils.run_bass_kernel_spmd(
        nc, in_maps, core_ids=list(range(N_CORES)), **kw)

    full = np.empty((B, S, E), dtype=np.float32)
    for c in range(N_CORES):
        b, h = divmod(c, 2)
        # out [128, 8, E] -> [SQ, E]
        o = res.results[c]["out"]
        full[b, h * SQ:(h + 1) * SQ, :] = o.transpose(1, 0, 2).reshape(SQ, E)
    kernel.last_results = res
    return full
